# revision 1
# baseline (speedup 1.0000x reference)
"""MoE routing kernel for Trainium2 (8 NeuronCores, expert-parallel).

Strategy (per spec sharding_hint):
  - Host computes the tiny gating Dense + softmax + top-2 routing (0.02% of
    the FLOPs) in float64 -- this decides the sharding, so it must run on
    host before dispatch.
  - Tokens are dispatched to expert-owning cores: core e receives the tokens
    whose top-2 includes expert e, pre-scaled by their combine weight and
    laid out transposed/tiled so the device DMA is fully contiguous.
  - Each core runs one dense [cap x 2048] @ [2048 x 2048] matmul against its
    resident expert weight (float32r on the PE at full rate, fp32 accuracy
    class), streaming token tiles, with weights loaded to SBUF exactly once.
  - Host scatters per-expert outputs back (y[idx] += Y_e) and adds the
    combine-weighted bias term.
"""

import numpy as np

N_TOKENS = 8192
D_IN = 2048
HIDDEN = 2048
NUM_EXPERTS = 8
TOP_K = 2
P = 128
NFREE = 512  # matmul moving free dim (one PSUM bank of fp32)

_KERNEL_CACHE: dict[int, object] = {}
LAST_EXEC_NS = None
LAST_TRACE = None


def _build_bass_kernel(cap: int):
    """Build + schedule the per-core Bass program for capacity `cap` tokens."""
    import concourse.bacc as bacc
    import concourse.tile as tile
    import concourse.mybir as mybir

    KO = D_IN // P        # 16 contraction tiles
    MT = cap // P         # token tiles
    NT = HIDDEN // NFREE  # 4 output column chunks

    nc = bacc.Bacc("TRN2", target_bir_lowering=False, debug=False)

    # xT layout: [ki, m_tile, ko, mi]  (value = xg[m*128+mi, ko*128+ki])
    xT = nc.dram_tensor("xT", [P, MT, KO, P], mybir.dt.float32r, kind="ExternalInput")
    w = nc.dram_tensor("w", [D_IN, HIDDEN], mybir.dt.float32r, kind="ExternalInput")
    y = nc.dram_tensor("y", [cap, HIDDEN], mybir.dt.float32, kind="ExternalOutput")

    # DMA chunking: whole-tile deps gate the first matmul, so W ko-rows are
    # split into 4 column tiles (256 KB each) and x m-tiles into 16 ko-chunks
    # (64 KB) — cuts PE start latency from one 1 MB single-queue transfer
    # (~44 us) to ~11 us. First-needed tiles (x m=0, W ko=0) are emitted
    # first so their DMAs run ahead of the bulk weight load.
    W_SPLIT, X_SPLIT = 4, 16
    WC, XC = HIDDEN // W_SPLIT, KO // X_SPLIT

    with tile.TileContext(nc) as tc:
        with (
            tc.tile_pool(name="wpool", bufs=1) as wpool,
            tc.tile_pool(name="xpool", bufs=3) as xpool,
            tc.tile_pool(name="opool", bufs=6) as opool,
            tc.tile_pool(name="psum", bufs=2, space="PSUM") as psum_pool,
        ):
            def load_x(m):
                xm = {}
                for c in range(X_SPLIT):
                    xt = xpool.tile([P, XC, P], mybir.dt.float32r,
                                    tag=f"xm{c}", name=f"x_{m}_{c}")
                    nc.sync.dma_start(out=xt[:], in_=xT[:, m, c * XC:(c + 1) * XC, :])
                    xm[c] = xt
                return xm

            w_k = {}

            def load_w(ko):
                for c in range(W_SPLIT):
                    wt = wpool.tile([P, WC], mybir.dt.float32r,
                                    tag=f"w{ko}_{c}", name=f"w_{ko}_{c}")
                    nc.sync.dma_start(out=wt[:], in_=w[ko * P:(ko + 1) * P,
                                                      c * WC:(c + 1) * WC])
                    w_k[ko, c] = wt

            x0 = load_x(0)
            for ko in range(KO):
                load_w(ko)

            def wslice(ko, n):
                c, r = divmod(n * NFREE, WC)
                return w_k[ko, c][:, r:r + NFREE]

            for m in range(MT):
                xm = x0 if m == 0 else load_x(m)
                # ko outer / n inner: the stationary operand (xm ko-slice)
                # stays loaded across the NT consecutive matmuls.
                ps = [
                    psum_pool.tile([P, NFREE], mybir.dt.float32,
                                   tag=f"ps{n}", name=f"ps_{m}_{n}")
                    for n in range(NT)
                ]
                for ko in range(KO):
                    xc, xr = divmod(ko, XC)
                    for n in range(NT):
                        nc.tensor.matmul(
                            ps[n][:],
                            lhsT=xm[xc][:, xr, :],
                            rhs=wslice(ko, n),
                            start=(ko == 0),
                            stop=(ko == KO - 1),
                        )
                for n in range(NT):
                    ot = opool.tile([P, NFREE], mybir.dt.float32,
                                    tag="ot", name=f"o_{m}_{n}")
                    nc.vector.tensor_copy(out=ot[:], in_=ps[n][:])
                    nc.sync.dma_start(
                        out=y[m * P:(m + 1) * P, n * NFREE:(n + 1) * NFREE],
                        in_=ot[:],
                    )

    nc.compile()
    return nc


def _route(x, Wg, bg):
    """Host gating in float64: softmax + top-2 (ties -> lower index, matching
    jax.lax.top_k)."""
    logits = x.astype(np.float64) @ Wg.astype(np.float64) + bg.astype(np.float64)
    logits -= logits.max(axis=-1, keepdims=True)
    p = np.exp(logits)
    p /= p.sum(axis=-1, keepdims=True)
    order = np.argsort(-p, axis=-1, kind="stable")
    top_idx = order[:, :TOP_K]                      # [N, K]
    top_w = np.take_along_axis(p, top_idx, axis=-1)  # [N, K]
    return top_idx, top_w.astype(np.float32)


def kernel(x, Wg, bg, W, b):
    x = np.asarray(x, dtype=np.float32)
    Wg = np.asarray(Wg, dtype=np.float32)
    bg = np.asarray(bg, dtype=np.float32)
    W = np.asarray(W, dtype=np.float32)
    b = np.asarray(b, dtype=np.float32)

    top_idx, top_w = _route(x, Wg, bg)

    # Per-expert token lists (an expert appears at most once per token).
    idx_e = []
    wgt_e = []
    for e in range(NUM_EXPERTS):
        hit = (top_idx == e)                        # [N, K] bool
        rows = np.nonzero(hit.any(axis=1))[0]
        wts = (top_w * hit).sum(axis=1)[rows].astype(np.float32)
        idx_e.append(rows)
        wgt_e.append(wts)

    counts = np.array([len(r) for r in idx_e])
    cap = max(P, int(-(-counts.max() // P)) * P)

    # The trimmed container lacks antenv.axon_hooks; stub it so a BASS_TRACE
    # request degrades to an untraced run instead of crashing.
    try:
        import antenv.axon_hooks  # noqa: F401
    except ImportError:
        import sys as _sys
        import types as _types

        _m = _types.ModuleType("antenv.axon_hooks")
        _m.get_axon_ntff_profile_hook = lambda: None
        _sys.modules["antenv.axon_hooks"] = _m

    from concourse import bass_utils

    nc = _KERNEL_CACHE.get(cap)
    if nc is None:
        nc = _build_bass_kernel(cap)
        _KERNEL_CACHE[cap] = nc

    KO = D_IN // P
    MT = cap // P

    in_maps = []
    for e in range(NUM_EXPERTS):
        xg = np.zeros((cap, D_IN), dtype=np.float32)
        xg[: counts[e]] = x[idx_e[e]] * wgt_e[e][:, None]
        # [cap, D] -> [ki, m_tile, ko, mi]
        xT = np.ascontiguousarray(
            xg.reshape(MT, P, KO, P).transpose(3, 0, 2, 1)
        )
        in_maps.append({"xT": xT, "w": np.ascontiguousarray(W[e])})

    import time as _time

    _t0 = _time.time()
    res = bass_utils.run_bass_kernel_spmd(
        nc, in_maps, core_ids=list(range(NUM_EXPERTS))
    )
    global LAST_EXEC_NS, LAST_TRACE, LAST_RUN_S
    LAST_RUN_S = _time.time() - _t0
    LAST_EXEC_NS = res.exec_time_ns
    LAST_TRACE = res.instructions_and_trace

    # Host combine: scatter-add expert outputs + combine-weighted bias.
    y = np.zeros((N_TOKENS, HIDDEN), dtype=np.float32)
    for e in range(NUM_EXPERTS):
        ye = res.results[e]["y"]
        y[idx_e[e]] += ye[: counts[e]]
        y[idx_e[e]] += wgt_e[e][:, None] * b[e][None, :]
    return y



# revision 4
# speedup vs baseline: 2.8659x; 2.8659x over previous
"""MoE routing kernel for Trainium2 (8 NeuronCores, expert-parallel).

The wall-clock of run_bass_kernel_spmd under the axon tunnel is dominated by
host<->device transfer (~25-60 MB/s), so the design ships the information-
theoretic minimum bytes and moves the token routing entirely on-device:

  - Host computes the tiny gating Dense + softmax + top-2 routing in float64
    (0.02% of the FLOPs) and builds the [N, E] combine matrix.
  - x is token-sharded: core c receives only its 1024-token block, transposed
    and cast to bf16 (4.2 MB/core; 33.5 MB total instead of 2x-duplicated
    expert-gathered fp32 tokens).
  - W is expert-sharded: core e holds expert e's weight in bf16 (8.4 MB/core).
  - On device: AllGather(x) over NeuronLink -> each core computes its expert's
    output for ALL 8192 tokens (dense [8192x2048]@[2048x2048] bf16 matmul,
    fp32 PSUM) -> scales each token row by that expert's combine weight ->
    ReduceScatter(add) -> core c ends with the final y for its token block.
  - y returns as bf16 [1024, 2048] per core; host upcasts and adds the
    combine-weighted bias term.

Computing all 8 experts for all tokens is 4x the minimal FLOPs but device
compute is ~1 ms vs ~3 s of tunnel transfer -- the transfer floor is what
matters: H2D = x(33.5) + W(67) + donated-output-zeros(33.5) MB, D2H = 33.5 MB.
"""

import numpy as np
from ml_dtypes import bfloat16

N_TOKENS = 8192
D_IN = 2048
HIDDEN = 2048
NUM_EXPERTS = 8
TOP_K = 2
P = 128
NFREE = 512  # matmul moving free dim (one PSUM bank of fp32)
TB = N_TOKENS // NUM_EXPERTS  # 1024 tokens per core

_KERNEL_CACHE: dict[str, object] = {}
LAST_EXEC_NS = None
LAST_TRACE = None
LAST_RUN_S = None


def _build_bass_kernel():
    """Per-core Bass program: AllGather(x) -> dense expert matmul -> combine
    scale -> ReduceScatter(y). Fixed shapes -- routing never changes them."""
    import concourse.bacc as bacc
    import concourse.tile as tile
    import concourse.mybir as mybir

    KO = D_IN // P          # 16 contraction tiles
    MT = TB // P            # 8 token tiles per block
    NT = HIDDEN // NFREE    # 4 output column chunks
    E = NUM_EXPERTS
    GROUPS = [list(range(E))]

    nc = bacc.Bacc("TRN2", target_bir_lowering=False, debug=False,
                   num_devices=E)

    # xT: this core's token block, transposed to [d, token] so the PE lhsT
    # (stationary) tiles slice directly.
    xT = nc.dram_tensor("xT", [D_IN, TB], mybir.dt.bfloat16, kind="ExternalInput")
    w = nc.dram_tensor("w", [D_IN, HIDDEN], mybir.dt.bfloat16, kind="ExternalInput")
    # cv[mi, e*MT+m] = combine weight of THIS core's expert for global token
    # (e*MT+m)*128 + mi.
    cv = nc.dram_tensor("cv", [P, E * MT], mybir.dt.float32, kind="ExternalInput")
    y = nc.dram_tensor("y", [TB, HIDDEN], mybir.dt.bfloat16, kind="ExternalOutput")

    with tile.TileContext(nc) as tc:
        with (
            tc.tile_pool(name="dram", bufs=1, space="DRAM") as dram,
            tc.tile_pool(name="wpool", bufs=1) as wpool,
            tc.tile_pool(name="cvpool", bufs=1) as cvpool,
            tc.tile_pool(name="xpool", bufs=2) as xpool,
            tc.tile_pool(name="opool", bufs=6) as opool,
            tc.tile_pool(name="psum", bufs=2, space="PSUM") as psum_pool,
        ):
            # --- collective dispatch: gather all cores' token blocks ---
            xb = dram.tile([D_IN, TB], mybir.dt.bfloat16, tag="xb", name="xb")
            xg = dram.tile([E, D_IN, TB], mybir.dt.bfloat16, tag="xg", name="xg")
            yfull = dram.tile([N_TOKENS, HIDDEN], mybir.dt.bfloat16,
                              tag="yfull", name="yfull")
            ys = dram.tile([TB, HIDDEN], mybir.dt.bfloat16, tag="ys", name="ys")

            nc.gpsimd.dma_start(out=xb[:], in_=xT[:])
            nc.gpsimd.collective_compute(
                "AllGather",
                mybir.AluOpType.bypass,
                replica_groups=GROUPS,
                ins=[xb.opt()],
                outs=[xg.opt()],
            )

            # --- resident weights + combine column (overlap the collective) ---
            w_k = []
            for ko in range(KO):
                wt = wpool.tile([P, HIDDEN], mybir.dt.bfloat16,
                                tag=f"w{ko}", name=f"w_{ko}")
                nc.sync.dma_start(out=wt[:], in_=w[ko * P:(ko + 1) * P, :])
                w_k.append(wt)
            cvt = cvpool.tile([P, E * MT], mybir.dt.float32, tag="cv", name="cvt")
            nc.sync.dma_start(out=cvt[:], in_=cv[:])

            # --- dense per-expert compute over every token block ---
            for e in range(E):
                xk = []
                for ki in range(KO):
                    xt = xpool.tile([P, TB], mybir.dt.bfloat16,
                                    tag=f"x{ki}", name=f"x_{e}_{ki}")
                    nc.sync.dma_start(out=xt[:], in_=xg[e, ki * P:(ki + 1) * P, :])
                    xk.append(xt)
                for m in range(MT):
                    ps = [
                        psum_pool.tile([P, NFREE], mybir.dt.float32,
                                       tag=f"ps{n}", name=f"ps_{e}_{m}_{n}")
                        for n in range(NT)
                    ]
                    for ki in range(KO):
                        for n in range(NT):
                            nc.tensor.matmul(
                                ps[n][:],
                                lhsT=xk[ki][:, m * P:(m + 1) * P],
                                rhs=w_k[ki][:, n * NFREE:(n + 1) * NFREE],
                                start=(ki == 0),
                                stop=(ki == KO - 1),
                            )
                    row = e * TB + m * P
                    col = e * MT + m
                    for n in range(NT):
                        ot = opool.tile([P, NFREE], mybir.dt.bfloat16,
                                        tag="ot", name=f"o_{e}_{m}_{n}")
                        nc.vector.tensor_scalar_mul(
                            ot[:], ps[n][:], cvt[:, col:col + 1])
                        nc.sync.dma_start(
                            out=yfull[row:row + P, n * NFREE:(n + 1) * NFREE],
                            in_=ot[:],
                        )

            # --- combine across experts, land own token block ---
            nc.gpsimd.collective_compute(
                "ReduceScatter",
                mybir.AluOpType.add,
                replica_groups=GROUPS,
                ins=[yfull.opt()],
                outs=[ys.opt()],
            )
            nc.gpsimd.dma_start(out=y[:], in_=ys[:])

    nc.compile()
    return nc


def _route(x, Wg, bg):
    """Host gating in float64: softmax + top-2 (ties -> lower index, matching
    jax.lax.top_k)."""
    logits = x.astype(np.float64) @ Wg.astype(np.float64) + bg.astype(np.float64)
    logits -= logits.max(axis=-1, keepdims=True)
    p = np.exp(logits)
    p /= p.sum(axis=-1, keepdims=True)
    order = np.argsort(-p, axis=-1, kind="stable")
    top_idx = order[:, :TOP_K]                      # [N, K]
    top_w = np.take_along_axis(p, top_idx, axis=-1)  # [N, K]
    return top_idx, top_w.astype(np.float32)


def kernel(x, Wg, bg, W, b):
    x = np.asarray(x, dtype=np.float32)
    Wg = np.asarray(Wg, dtype=np.float32)
    bg = np.asarray(bg, dtype=np.float32)
    W = np.asarray(W, dtype=np.float32)
    b = np.asarray(b, dtype=np.float32)

    top_idx, top_w = _route(x, Wg, bg)
    combine = np.zeros((N_TOKENS, NUM_EXPERTS), dtype=np.float32)
    np.put_along_axis(combine, top_idx, top_w, axis=-1)

    # The trimmed container lacks antenv.axon_hooks; stub it so a BASS_TRACE
    # request degrades to an untraced run instead of crashing.
    try:
        import antenv.axon_hooks  # noqa: F401
    except ImportError:
        import sys as _sys
        import types as _types

        _m = _types.ModuleType("antenv.axon_hooks")
        _m.get_axon_ntff_profile_hook = lambda: None
        _sys.modules["antenv.axon_hooks"] = _m

    from concourse import bass_utils

    nc = _KERNEL_CACHE.get("nc")
    if nc is None:
        nc = _build_bass_kernel()
        _KERNEL_CACHE["nc"] = nc

    MT = TB // P
    in_maps = []
    for c in range(NUM_EXPERTS):
        xT = np.ascontiguousarray(
            x[c * TB:(c + 1) * TB].T).astype(bfloat16)          # [D, TB]
        cvt = np.ascontiguousarray(
            combine[:, c].reshape(NUM_EXPERTS * MT, P).T)
        in_maps.append({
            "xT": xT,
            "w": W[c].astype(bfloat16),
            "cv": cvt,
        })

    import time as _time

    _t0 = _time.time()
    res = bass_utils.run_bass_kernel_spmd(
        nc, in_maps, core_ids=list(range(NUM_EXPERTS))
    )
    global LAST_EXEC_NS, LAST_TRACE, LAST_RUN_S
    LAST_RUN_S = _time.time() - _t0
    LAST_EXEC_NS = res.exec_time_ns
    LAST_TRACE = res.instructions_and_trace

    # Host epilogue: upcast, add combine-weighted bias.
    y = np.concatenate(
        [res.results[c]["y"] for c in range(NUM_EXPERTS)], axis=0
    ).astype(np.float32)
    y += combine @ b
    return y


# revision 6
# speedup vs baseline: 5.0214x; 1.7522x over previous
"""MoE routing kernel for Trainium2 (8 NeuronCores, expert-parallel).

The wall-clock of run_bass_kernel_spmd under the axon tunnel is dominated by
host<->device transfer (~45 MB/s H2D, ~35 MB/s D2H), so the design ships the
minimum bytes and moves the token routing entirely on-device:

  - Host computes the tiny gating Dense + softmax + top-2 routing in float64
    (0.02% of the FLOPs) and builds the [N, E] combine matrix.
  - x is token-sharded and int8-quantized per (feature, block) with fp32
    scales (2.1 MB/core; 16.8 MB total).
  - W is expert-sharded and int8-quantized per feature row (4.2 MB/core).
  - On device: AllGather(x int8) over NeuronLink -> dequant to bf16 ->
    each core computes its expert's output for ALL 8192 tokens (dense bf16
    matmul, fp32 PSUM) -> scales rows by the expert's combine weight (fp32)
    -> ReduceScatter(add, fp32) -> core c holds the exact fp32 y for its
    token block -> per-token abs-max int8 quantization on device.
  - y returns as int8 [1024, 2048] + fp32 per-token scales; host dequants
    and adds the combine-weighted bias term.

Computing all 8 experts for all tokens is 4x the minimal FLOPs but device
compute is ~1 ms vs ~2 s of tunnel transfer. Error budget (validated by
simulation exactly matching HW for the bf16 variant): x-int8 0.75% +
W-int8 0.76% + y-int8 0.8% -> fro rel err ~1.43e-2, under the 2e-2 gate.
"""

import numpy as np

N_TOKENS = 8192
D_IN = 2048
HIDDEN = 2048
NUM_EXPERTS = 8
TOP_K = 2
P = 128
NFREE = 512  # matmul moving free dim (one PSUM bank of fp32)
TB = N_TOKENS // NUM_EXPERTS  # 1024 tokens per core

_KERNEL_CACHE: dict[str, object] = {}
LAST_EXEC_NS = None
LAST_TRACE = None
LAST_RUN_S = None


def _build_bass_kernel():
    """Per-core Bass program: AllGather(x) -> dequant -> dense expert matmul
    -> combine scale -> ReduceScatter -> int8 quantize. Fixed shapes --
    routing never changes them."""
    import concourse.bacc as bacc
    import concourse.tile as tile
    import concourse.mybir as mybir

    KO = D_IN // P          # 16 contraction tiles
    MT = TB // P            # 8 token tiles per block
    NT = HIDDEN // NFREE    # 4 output column chunks
    E = NUM_EXPERTS
    GROUPS = [list(range(E))]

    nc = bacc.Bacc("TRN2", target_bir_lowering=False, debug=False,
                   num_devices=E)

    # xq: this core's token block, transposed to [d, token] (so PE lhsT tiles
    # slice directly), int8 with per-(d, block) scales.
    xq = nc.dram_tensor("xq", [D_IN, TB], mybir.dt.int8, kind="ExternalInput")
    # sxg[p, e*KO+ki] = dequant scale for feature row d=ki*128+p of block e
    # (identical on every core -- each core dequants all gathered blocks).
    sxg = nc.dram_tensor("sxg", [P, E * KO], mybir.dt.float32, kind="ExternalInput")
    wq = nc.dram_tensor("wq", [D_IN, HIDDEN], mybir.dt.int8, kind="ExternalInput")
    ws = nc.dram_tensor("ws", [P, KO], mybir.dt.float32, kind="ExternalInput")
    # cv[mi, e*MT+m] = combine weight of THIS core's expert for global token
    # (e*MT+m)*128 + mi.
    cv = nc.dram_tensor("cv", [P, E * MT], mybir.dt.float32, kind="ExternalInput")
    yq = nc.dram_tensor("yq", [TB, HIDDEN], mybir.dt.int8, kind="ExternalOutput")
    ysc = nc.dram_tensor("ysc", [P, MT], mybir.dt.float32, kind="ExternalOutput")

    with tile.TileContext(nc) as tc:
        with (
            tc.tile_pool(name="dram", bufs=1, space="DRAM") as dram,
            tc.tile_pool(name="wpool", bufs=1) as wpool,
            tc.tile_pool(name="qpool", bufs=1) as qpool,
            tc.tile_pool(name="scpool", bufs=1) as scpool,
            tc.tile_pool(name="xpool", bufs=2) as xpool,
            tc.tile_pool(name="xqpool", bufs=1) as xqpool,
            tc.tile_pool(name="opool", bufs=6) as opool,
            tc.tile_pool(name="ypool", bufs=1) as ypool,
            tc.tile_pool(name="psum", bufs=2, space="PSUM") as psum_pool,
        ):
            # --- collective dispatch: gather all cores' int8 token blocks ---
            xb = dram.tile([D_IN, TB], mybir.dt.int8, tag="xb", name="xb")
            xg = dram.tile([E, D_IN, TB], mybir.dt.int8, tag="xg", name="xg")
            yfull = dram.tile([N_TOKENS, HIDDEN], mybir.dt.float32,
                              tag="yfull", name="yfull")
            ys = dram.tile([TB, HIDDEN], mybir.dt.float32, tag="ys", name="ys")

            nc.gpsimd.dma_start(out=xb[:], in_=xq[:])
            nc.gpsimd.collective_compute(
                "AllGather",
                mybir.AluOpType.bypass,
                replica_groups=GROUPS,
                ins=[xb.opt()],
                outs=[xg.opt()],
            )

            # --- scales + combine weights ---
            sxt = scpool.tile([P, E * KO], mybir.dt.float32, tag="sx", name="sxt")
            nc.sync.dma_start(out=sxt[:], in_=sxg[:])
            wst = scpool.tile([P, KO], mybir.dt.float32, tag="ws", name="wst")
            nc.sync.dma_start(out=wst[:], in_=ws[:])
            cvt = scpool.tile([P, E * MT], mybir.dt.float32, tag="cv", name="cvt")
            nc.sync.dma_start(out=cvt[:], in_=cv[:])

            # --- resident weights: int8 -> bf16 dequant (overlaps AllGather) ---
            w_k = []
            for ko in range(KO):
                qt = qpool.tile([P, HIDDEN], mybir.dt.int8,
                                tag=f"wq{ko % 2}", name=f"wq_{ko}")
                nc.sync.dma_start(out=qt[:], in_=wq[ko * P:(ko + 1) * P, :])
                wt = wpool.tile([P, HIDDEN], mybir.dt.bfloat16,
                                tag=f"w{ko}", name=f"w_{ko}")
                nc.vector.tensor_scalar_mul(wt[:], qt[:], wst[:, ko:ko + 1])
                w_k.append(wt)

            # --- dense per-expert compute over every gathered block ---
            for e in range(E):
                xk = []
                for ki in range(KO):
                    xqt = xqpool.tile([P, TB], mybir.dt.int8,
                                      tag=f"xq{ki % 4}", name=f"xq_{e}_{ki}")
                    nc.sync.dma_start(out=xqt[:], in_=xg[e, ki * P:(ki + 1) * P, :])
                    xt = xpool.tile([P, TB], mybir.dt.bfloat16,
                                    tag=f"x{ki}", name=f"x_{e}_{ki}")
                    nc.vector.tensor_scalar_mul(
                        xt[:], xqt[:], sxt[:, e * KO + ki:e * KO + ki + 1])
                    xk.append(xt)
                for m in range(MT):
                    ps = [
                        psum_pool.tile([P, NFREE], mybir.dt.float32,
                                       tag=f"ps{n}", name=f"ps_{e}_{m}_{n}")
                        for n in range(NT)
                    ]
                    for ki in range(KO):
                        for n in range(NT):
                            nc.tensor.matmul(
                                ps[n][:],
                                lhsT=xk[ki][:, m * P:(m + 1) * P],
                                rhs=w_k[ki][:, n * NFREE:(n + 1) * NFREE],
                                start=(ki == 0),
                                stop=(ki == KO - 1),
                            )
                    row = e * TB + m * P
                    col = e * MT + m
                    for n in range(NT):
                        ot = opool.tile([P, NFREE], mybir.dt.float32,
                                        tag="ot", name=f"o_{e}_{m}_{n}")
                        nc.vector.tensor_scalar_mul(
                            ot[:], ps[n][:], cvt[:, col:col + 1])
                        nc.sync.dma_start(
                            out=yfull[row:row + P, n * NFREE:(n + 1) * NFREE],
                            in_=ot[:],
                        )

            # --- combine across experts, land own token block (fp32) ---
            nc.gpsimd.collective_compute(
                "ReduceScatter",
                mybir.AluOpType.add,
                replica_groups=GROUPS,
                ins=[yfull.opt()],
                outs=[ys.opt()],
            )

            # --- per-token abs-max int8 quantization ---
            sct = scpool.tile([P, MT], mybir.dt.float32, tag="sct", name="sct")
            for m in range(MT):
                yt = ypool.tile([P, HIDDEN], mybir.dt.float32,
                                tag=f"yt{m % 2}", name=f"yt_{m}")
                nc.sync.dma_start(out=yt[:], in_=ys[m * P:(m + 1) * P, :])
                amax = scpool.tile([P, 1], mybir.dt.float32,
                                   tag=f"am{m % 2}", name=f"amax_{m}")
                nc.vector.tensor_reduce(
                    amax[:], yt[:], axis=mybir.AxisListType.XYZW,
                    op=mybir.AluOpType.max, apply_absolute_value=True)
                nc.vector.tensor_scalar_max(amax[:], amax[:], 1e-30)
                rq = scpool.tile([P, 1], mybir.dt.float32,
                                 tag=f"rq{m % 2}", name=f"rq_{m}")
                nc.vector.reciprocal(rq[:], amax[:])
                nc.vector.tensor_scalar_mul(rq[:], rq[:], 127.0)
                nc.vector.tensor_scalar_mul(
                    sct[:, m:m + 1], amax[:], 1.0 / 127.0)
                qt = ypool.tile([P, HIDDEN], mybir.dt.int8,
                                tag=f"q{m % 2}", name=f"q_{m}")
                nc.vector.tensor_scalar_mul(qt[:], yt[:], rq[:])
                nc.sync.dma_start(out=yq[m * P:(m + 1) * P, :], in_=qt[:])
            nc.sync.dma_start(out=ysc[:], in_=sct[:])

    nc.compile()
    return nc


def _route(x, Wg, bg):
    """Host gating in float64: softmax + top-2 (ties -> lower index, matching
    jax.lax.top_k)."""
    logits = x.astype(np.float64) @ Wg.astype(np.float64) + bg.astype(np.float64)
    logits -= logits.max(axis=-1, keepdims=True)
    p = np.exp(logits)
    p /= p.sum(axis=-1, keepdims=True)
    order = np.argsort(-p, axis=-1, kind="stable")
    top_idx = order[:, :TOP_K]                      # [N, K]
    top_w = np.take_along_axis(p, top_idx, axis=-1)  # [N, K]
    return top_idx, top_w.astype(np.float32)


def _quant_rows(a):
    """int8 symmetric quantization along axis -1; returns (q, scale)."""
    s = np.abs(a).max(axis=-1) / 127.0
    s[s == 0] = 1.0
    q = np.clip(np.rint(a / s[..., None]), -127, 127).astype(np.int8)
    return q, s.astype(np.float32)


def kernel(x, Wg, bg, W, b):
    x = np.asarray(x, dtype=np.float32)
    Wg = np.asarray(Wg, dtype=np.float32)
    bg = np.asarray(bg, dtype=np.float32)
    W = np.asarray(W, dtype=np.float32)
    b = np.asarray(b, dtype=np.float32)

    top_idx, top_w = _route(x, Wg, bg)
    combine = np.zeros((N_TOKENS, NUM_EXPERTS), dtype=np.float32)
    np.put_along_axis(combine, top_idx, top_w, axis=-1)

    # The trimmed container lacks antenv.axon_hooks; stub it so a BASS_TRACE
    # request degrades to an untraced run instead of crashing.
    try:
        import antenv.axon_hooks  # noqa: F401
    except ImportError:
        import sys as _sys
        import types as _types

        _m = _types.ModuleType("antenv.axon_hooks")
        _m.get_axon_ntff_profile_hook = lambda: None
        _sys.modules["antenv.axon_hooks"] = _m

    from concourse import bass_utils

    nc = _KERNEL_CACHE.get("nc")
    if nc is None:
        nc = _build_bass_kernel()
        _KERNEL_CACHE["nc"] = nc

    KO = D_IN // P
    MT = TB // P

    # Quantize x per (feature, block): block c rows are x[c*TB:(c+1)*TB].T.
    xq_blocks = []
    sx = np.empty((D_IN, NUM_EXPERTS), dtype=np.float32)
    for c in range(NUM_EXPERTS):
        blk = np.ascontiguousarray(x[c * TB:(c + 1) * TB].T)   # [D, TB]
        q, s = _quant_rows(blk)
        xq_blocks.append(q)
        sx[:, c] = s
    # sxg[p, e*KO+ki] = sx[ki*128+p, e] -- same for every core.
    sxg = np.ascontiguousarray(
        sx.reshape(KO, P, NUM_EXPERTS).transpose(1, 2, 0).reshape(P, -1))

    in_maps = []
    for c in range(NUM_EXPERTS):
        wq, sw = _quant_rows(W[c])                              # [D, H], [D]
        cvt = np.ascontiguousarray(
            combine[:, c].reshape(NUM_EXPERTS * MT, P).T)
        in_maps.append({
            "xq": xq_blocks[c],
            "sxg": sxg,
            "wq": wq,
            "ws": np.ascontiguousarray(sw.reshape(KO, P).T),
            "cv": cvt,
        })

    import time as _time

    _t0 = _time.time()
    res = bass_utils.run_bass_kernel_spmd(
        nc, in_maps, core_ids=list(range(NUM_EXPERTS))
    )
    global LAST_EXEC_NS, LAST_TRACE, LAST_RUN_S
    LAST_RUN_S = _time.time() - _t0
    LAST_EXEC_NS = res.exec_time_ns
    LAST_TRACE = res.instructions_and_trace

    # Host epilogue: dequant, add combine-weighted bias.
    y = np.empty((N_TOKENS, HIDDEN), dtype=np.float32)
    for c in range(NUM_EXPERTS):
        q = res.results[c]["yq"].astype(np.float32)             # [TB, H]
        s = res.results[c]["ysc"].T.reshape(TB, 1)              # [TB, 1]
        y[c * TB:(c + 1) * TB] = q * s
    y += combine @ b
    return y


# revision 7
# speedup vs baseline: 5.4838x; 1.0921x over previous
"""MoE routing kernel for Trainium2 (8 NeuronCores, expert-parallel).

The wall-clock of run_bass_kernel_spmd under the axon tunnel is dominated by
host<->device transfer (~45 MB/s H2D, ~35 MB/s D2H), so the design ships the
minimum bytes and moves the token routing entirely on-device:

  - Host computes the tiny gating Dense + softmax + top-2 routing in float64
    (0.02% of the FLOPs) and builds the [N, E] combine matrix.
  - x is token-sharded and int8-quantized per (feature, block) with fp32
    scales (2.1 MB/core; 16.8 MB total).
  - W is expert-sharded and int8-quantized per feature row (4.2 MB/core).
  - On device: AllGather(x int8) over NeuronLink -> dequant to bf16 ->
    each core computes its expert's output for ALL 8192 tokens (dense bf16
    matmul, fp32 PSUM) -> scales rows by the expert's combine weight (fp32)
    -> ReduceScatter(add, fp32) -> core c holds the exact fp32 y for its
    token block -> per-token abs-max int8 quantization on device.
  - y returns as int8 [1024, 2048] with fp32 per-token scales bit-packed
    into two extra int8 rows; host dequants and adds the combine-weighted
    bias term.

All per-core tensors are merged into one int8 input, one fp32 scale input
and one int8 output to minimize per-tensor transfer overhead. Computing all
8 experts for all tokens is 4x the minimal FLOPs but device compute is ~1 ms
vs ~2 s of tunnel transfer. Error budget (validated by simulation that
matches HW to 4 digits): x-int8 0.75% + W-int8 0.76% + y-int8 0.8% ->
fro rel err ~1.43e-2, under the 2e-2 gate.
"""

import numpy as np

N_TOKENS = 8192
D_IN = 2048
HIDDEN = 2048
NUM_EXPERTS = 8
TOP_K = 2
P = 128
NFREE = 512  # matmul moving free dim (one PSUM bank of fp32)
TB = N_TOKENS // NUM_EXPERTS  # 1024 tokens per core

_KERNEL_CACHE: dict[str, object] = {}
LAST_EXEC_NS = None
LAST_TRACE = None
LAST_RUN_S = None


def _build_bass_kernel():
    """Per-core Bass program: AllGather(x) -> dequant -> dense expert matmul
    -> combine scale -> ReduceScatter -> int8 quantize. Fixed shapes --
    routing never changes them."""
    import concourse.bacc as bacc
    import concourse.tile as tile
    import concourse.mybir as mybir

    KO = D_IN // P          # 16 contraction tiles
    MT = TB // P            # 8 token tiles per block
    NT = HIDDEN // NFREE    # 4 output column chunks
    E = NUM_EXPERTS
    GROUPS = [list(range(E))]
    NSC = E * KO + KO + E * MT  # sxg | ws | cv columns

    nc = bacc.Bacc("TRN2", target_bir_lowering=False, debug=False,
                   num_devices=E)

    # qin[:, :TB] = this core's token block, transposed to [d, token] int8;
    # qin[:, TB:] = this core's expert weight [d, h] int8.
    qin = nc.dram_tensor("qin", [D_IN, TB + HIDDEN], mybir.dt.int8,
                         kind="ExternalInput")
    # scl columns: [0,128) sxg (x dequant scales for all gathered blocks,
    # sxg[p, e*KO+ki] = scale of feature d=ki*128+p of block e, same on every
    # core); [128,144) ws (W dequant scales); [144,208) cv (combine weight of
    # THIS core's expert for token (e*MT+m)*128+mi at column e*MT+m).
    scl = nc.dram_tensor("scl", [P, NSC], mybir.dt.float32, kind="ExternalInput")
    # out rows [0,TB) = y int8; rows [TB,TB+2) = per-token fp32 scales,
    # bit-packed ([128 tokens-within-tile, MT] fp32 -> [128, 32] int8 bytes).
    out = nc.dram_tensor("out", [TB + 2, HIDDEN], mybir.dt.int8,
                         kind="ExternalOutput")

    with tile.TileContext(nc) as tc:
        with (
            tc.tile_pool(name="dram", bufs=1, space="DRAM") as dram,
            tc.tile_pool(name="wpool", bufs=1) as wpool,
            tc.tile_pool(name="qpool", bufs=1) as qpool,
            tc.tile_pool(name="scpool", bufs=1) as scpool,
            tc.tile_pool(name="xpool", bufs=2) as xpool,
            tc.tile_pool(name="xqpool", bufs=1) as xqpool,
            tc.tile_pool(name="opool", bufs=6) as opool,
            tc.tile_pool(name="ypool", bufs=1) as ypool,
            tc.tile_pool(name="psum", bufs=2, space="PSUM") as psum_pool,
        ):
            # --- collective dispatch: gather all cores' int8 token blocks ---
            xb = dram.tile([D_IN, TB], mybir.dt.int8, tag="xb", name="xb")
            xg = dram.tile([E, D_IN, TB], mybir.dt.int8, tag="xg", name="xg")
            yfull = dram.tile([N_TOKENS, HIDDEN], mybir.dt.float32,
                              tag="yfull", name="yfull")
            ys = dram.tile([TB, HIDDEN], mybir.dt.float32, tag="ys", name="ys")

            nc.gpsimd.dma_start(out=xb[:], in_=qin[:, :TB])
            nc.gpsimd.collective_compute(
                "AllGather",
                mybir.AluOpType.bypass,
                replica_groups=GROUPS,
                ins=[xb.opt()],
                outs=[xg.opt()],
            )

            # --- scales + combine weights (one DMA) ---
            sclt = scpool.tile([P, NSC], mybir.dt.float32, tag="sc", name="sclt")
            nc.sync.dma_start(out=sclt[:], in_=scl[:])
            sxt = sclt[:, 0:E * KO]
            wst = sclt[:, E * KO:E * KO + KO]
            cvt = sclt[:, E * KO + KO:NSC]

            # --- resident weights: int8 -> bf16 dequant (overlaps AllGather) ---
            w_k = []
            for ko in range(KO):
                qt = qpool.tile([P, HIDDEN], mybir.dt.int8,
                                tag=f"wq{ko % 2}", name=f"wq_{ko}")
                nc.sync.dma_start(out=qt[:], in_=qin[ko * P:(ko + 1) * P, TB:])
                wt = wpool.tile([P, HIDDEN], mybir.dt.bfloat16,
                                tag=f"w{ko}", name=f"w_{ko}")
                nc.vector.tensor_scalar_mul(wt[:], qt[:], wst[:, ko:ko + 1])
                w_k.append(wt)

            # --- dense per-expert compute over every gathered block ---
            for e in range(E):
                xk = []
                for ki in range(KO):
                    xqt = xqpool.tile([P, TB], mybir.dt.int8,
                                      tag=f"xq{ki % 4}", name=f"xq_{e}_{ki}")
                    nc.sync.dma_start(out=xqt[:], in_=xg[e, ki * P:(ki + 1) * P, :])
                    xt = xpool.tile([P, TB], mybir.dt.bfloat16,
                                    tag=f"x{ki}", name=f"x_{e}_{ki}")
                    nc.vector.tensor_scalar_mul(
                        xt[:], xqt[:], sxt[:, e * KO + ki:e * KO + ki + 1])
                    xk.append(xt)
                for m in range(MT):
                    ps = [
                        psum_pool.tile([P, NFREE], mybir.dt.float32,
                                       tag=f"ps{n}", name=f"ps_{e}_{m}_{n}")
                        for n in range(NT)
                    ]
                    for ki in range(KO):
                        for n in range(NT):
                            nc.tensor.matmul(
                                ps[n][:],
                                lhsT=xk[ki][:, m * P:(m + 1) * P],
                                rhs=w_k[ki][:, n * NFREE:(n + 1) * NFREE],
                                start=(ki == 0),
                                stop=(ki == KO - 1),
                            )
                    row = e * TB + m * P
                    col = e * MT + m
                    for n in range(NT):
                        ot = opool.tile([P, NFREE], mybir.dt.float32,
                                        tag="ot", name=f"o_{e}_{m}_{n}")
                        nc.vector.tensor_scalar_mul(
                            ot[:], ps[n][:], cvt[:, col:col + 1])
                        nc.sync.dma_start(
                            out=yfull[row:row + P, n * NFREE:(n + 1) * NFREE],
                            in_=ot[:],
                        )

            # --- combine across experts, land own token block (fp32) ---
            nc.gpsimd.collective_compute(
                "ReduceScatter",
                mybir.AluOpType.add,
                replica_groups=GROUPS,
                ins=[yfull.opt()],
                outs=[ys.opt()],
            )

            # --- per-token abs-max int8 quantization ---
            sct = scpool.tile([P, MT], mybir.dt.float32, tag="sct", name="sct")
            for m in range(MT):
                yt = ypool.tile([P, HIDDEN], mybir.dt.float32,
                                tag=f"yt{m % 2}", name=f"yt_{m}")
                nc.sync.dma_start(out=yt[:], in_=ys[m * P:(m + 1) * P, :])
                amax = scpool.tile([P, 1], mybir.dt.float32,
                                   tag=f"am{m % 2}", name=f"amax_{m}")
                nc.vector.tensor_reduce(
                    amax[:], yt[:], axis=mybir.AxisListType.XYZW,
                    op=mybir.AluOpType.max, apply_absolute_value=True)
                nc.vector.tensor_scalar_max(amax[:], amax[:], 1e-30)
                rq = scpool.tile([P, 1], mybir.dt.float32,
                                 tag=f"rq{m % 2}", name=f"rq_{m}")
                nc.vector.reciprocal(rq[:], amax[:])
                nc.vector.tensor_scalar_mul(rq[:], rq[:], 127.0)
                nc.vector.tensor_scalar_mul(
                    sct[:, m:m + 1], amax[:], 1.0 / 127.0)
                qt = ypool.tile([P, HIDDEN], mybir.dt.int8,
                                tag=f"q{m % 2}", name=f"q_{m}")
                nc.vector.tensor_scalar_mul(qt[:], yt[:], rq[:])
                nc.sync.dma_start(out=out[m * P:(m + 1) * P, :], in_=qt[:])
            # bit-pack the fp32 scales into the two trailing int8 rows
            nc.sync.dma_start(
                out=out[TB:TB + 2, :].flatten().rearrange("(p f) -> p f", p=P),
                in_=sct[:].bitcast(mybir.dt.int8),
            )

    nc.compile()
    return nc


def _route(x, Wg, bg):
    """Host gating in float64: softmax + top-2 (ties -> lower index, matching
    jax.lax.top_k)."""
    logits = x.astype(np.float64) @ Wg.astype(np.float64) + bg.astype(np.float64)
    logits -= logits.max(axis=-1, keepdims=True)
    p = np.exp(logits)
    p /= p.sum(axis=-1, keepdims=True)
    order = np.argsort(-p, axis=-1, kind="stable")
    top_idx = order[:, :TOP_K]                      # [N, K]
    top_w = np.take_along_axis(p, top_idx, axis=-1)  # [N, K]
    return top_idx, top_w.astype(np.float32)


def _quant_rows(a):
    """int8 symmetric quantization along axis -1; returns (q, scale)."""
    s = np.abs(a).max(axis=-1) / 127.0
    s[s == 0] = 1.0
    q = np.clip(np.rint(a / s[..., None]), -127, 127).astype(np.int8)
    return q, s.astype(np.float32)


def kernel(x, Wg, bg, W, b):
    x = np.asarray(x, dtype=np.float32)
    Wg = np.asarray(Wg, dtype=np.float32)
    bg = np.asarray(bg, dtype=np.float32)
    W = np.asarray(W, dtype=np.float32)
    b = np.asarray(b, dtype=np.float32)

    top_idx, top_w = _route(x, Wg, bg)
    combine = np.zeros((N_TOKENS, NUM_EXPERTS), dtype=np.float32)
    np.put_along_axis(combine, top_idx, top_w, axis=-1)

    # The trimmed container lacks antenv.axon_hooks; stub it so a BASS_TRACE
    # request degrades to an untraced run instead of crashing.
    try:
        import antenv.axon_hooks  # noqa: F401
    except ImportError:
        import sys as _sys
        import types as _types

        _m = _types.ModuleType("antenv.axon_hooks")
        _m.get_axon_ntff_profile_hook = lambda: None
        _sys.modules["antenv.axon_hooks"] = _m

    from concourse import bass_utils

    nc = _KERNEL_CACHE.get("nc")
    if nc is None:
        nc = _build_bass_kernel()
        _KERNEL_CACHE["nc"] = nc

    KO = D_IN // P
    MT = TB // P

    # Quantize x per (feature, block): block c rows are x[c*TB:(c+1)*TB].T.
    xq_blocks = []
    sx = np.empty((D_IN, NUM_EXPERTS), dtype=np.float32)
    for c in range(NUM_EXPERTS):
        blk = np.ascontiguousarray(x[c * TB:(c + 1) * TB].T)   # [D, TB]
        q, s = _quant_rows(blk)
        xq_blocks.append(q)
        sx[:, c] = s
    # sxg[p, e*KO+ki] = sx[ki*128+p, e] -- same for every core.
    sxg = np.ascontiguousarray(
        sx.reshape(KO, P, NUM_EXPERTS).transpose(1, 2, 0).reshape(P, -1))

    in_maps = []
    for c in range(NUM_EXPERTS):
        wq, sw = _quant_rows(W[c])                              # [D, H], [D]
        cvt = combine[:, c].reshape(NUM_EXPERTS * MT, P).T      # [P, E*MT]
        in_maps.append({
            "qin": np.concatenate([xq_blocks[c], wq], axis=1),
            "scl": np.concatenate(
                [sxg, sw.reshape(KO, P).T, cvt], axis=1).astype(np.float32),
        })

    import time as _time

    _t0 = _time.time()
    res = bass_utils.run_bass_kernel_spmd(
        nc, in_maps, core_ids=list(range(NUM_EXPERTS))
    )
    global LAST_EXEC_NS, LAST_TRACE, LAST_RUN_S
    LAST_RUN_S = _time.time() - _t0
    LAST_EXEC_NS = res.exec_time_ns
    LAST_TRACE = res.instructions_and_trace

    # Host epilogue: dequant, add combine-weighted bias.
    y = np.empty((N_TOKENS, HIDDEN), dtype=np.float32)
    for c in range(NUM_EXPERTS):
        o = res.results[c]["out"]
        q = o[:TB].astype(np.float32)                           # [TB, H]
        sct = o[TB:TB + 2].reshape(P, 4 * MT).view(np.float32)  # [P, MT]
        s = sct.T.reshape(TB, 1)                                # [TB, 1]
        y[c * TB:(c + 1) * TB] = q * s
    y += combine @ b
    return y
